# revision 1
# baseline (speedup 1.0000x reference)
"""Ragged grouped-GEMM (MoE group linear) on 8 trn2 NeuronCores.

y[s_g:e_g] = x[s_g:e_g] @ w[g].T  for 64 expert segments given by
cumulative offsets.

Strategy: token-shard 8192 tokens per core (perfectly balanced
compute).  On the host, split each core's token range into per-expert
segments and bin-pack each segment's 128-token tiles into
variable-size weight "slots" (8/4/2/1 tiles per slot, one expert
weight DMA per slot).  The slot inventory is computed from the actual
offsets at compile time and shared by all 8 cores, so a single static
SPMD program serves all cores; per-core raggedness lives entirely in
the data (which expert weight / which tokens each slot carries).
Matmuls run in bf16 with fp32 PSUM accumulation; the host
pre-transposes x tiles (in-major) and weights (in x out) so the device
does pure DMA + matmul + evict.
"""

import math
import os
import time
import numpy as np
import ml_dtypes

import concourse.bass as bass
import concourse.mybir as mybir
import concourse.tile as tile
from concourse import bacc
from concourse.bass_utils import run_bass_kernel_spmd

T_TOK = 65536
G_EXP = 64
DIN = 1024
DOUT = 1024
NCORES = 8
TPC = T_TOK // NCORES
TILE = 128
KTILES = DIN // 128
BF16 = mybir.dt.bfloat16
F32 = mybir.dt.float32
MAXSLOT = 8  # max tiles per weight slot

_COMPILED = {}
LAST_EXEC_NS = None
SPMD_WALL_S = None


def _segments(offs):
    """Per-core list of (expert, tok_start, ntiles)."""
    bounds = np.concatenate([[0], np.asarray(offs, dtype=np.int64)])
    per_core = []
    for c in range(NCORES):
        lo, hi = c * TPC, (c + 1) * TPC
        segs = []
        for g in range(G_EXP):
            s, e = max(int(bounds[g]), lo), min(int(bounds[g + 1]), hi)
            if e > s:
                segs.append((g, s, e, math.ceil((e - s) / TILE)))
        per_core.append(segs)
    return per_core


def _fit_core(segs, inv):
    """Greedy fit of segments onto slot multiset inv: per segment,
    smallest free slot >= remainder if any (pad), else largest free.
    Returns assignment [(slot_i, sz, g, base, e)] or None."""
    free = sorted(((sz, i) for i, sz in enumerate(inv)))
    assign = []
    for g, s, e, nt in sorted(segs, key=lambda t: -t[3]):
        rem = nt
        base = s
        while rem > 0:
            if not free:
                return None
            ge = [(sz, i) for sz, i in free if sz >= rem]
            sz, i = ge[0] if ge else free[-1]
            free.remove((sz, i))
            assign.append((i, sz, g, base, e))
            step = min(sz, rem)
            base += step * TILE
            rem -= step
    return assign


def _plan(offs):
    """Returns (slot_sizes, per-core fills). Shared slot inventory is
    chosen by a small parametric search minimizing max(PE, DMA) cost
    estimates; falls back to growing capacity until feasible."""
    per_core = _segments(offs)

    best = None
    for a8 in range(5, 13):
        for a6 in range(0, 4):
            for a4 in range(0, 6):
                for a2 in range(0, 6):
                    for a1 in range(0, 5):
                        inv = [8] * a8 + [6] * a6 + [4] * a4 \
                            + [2] * a2 + [1] * a1
                        if any(_fit_core(s, inv) is None for s in per_core):
                            continue
                        cap, ns = sum(inv), len(inv)
                        cost = max(3.4 * cap, (18 + 2 * ns + 0.5 * cap) * 2.79)
                        if best is None or cost < best[0]:
                            best = (cost, inv)
    if best is None:  # pathological offsets: grow until feasible
        inv = []
        while any(_fit_core(s, inv) is None for s in per_core):
            inv = sorted(inv + [8] * 4 + [2] * 2 + [1] * 2, reverse=True)
        best = (0, inv)
    slot_sizes = sorted(best[1], reverse=True)

    fills = []
    for c in range(NCORES):
        assign = _fit_core(per_core[c], slot_sizes)
        fill = [None] * len(slot_sizes)
        for i, sz, g, base, e in assign:
            rows = [max(0, min(TILE, e - (base + ti * TILE)))
                    for ti in range(sz)]
            fill[i] = (sz, g, base, rows)
        fills.append(fill)
    return slot_sizes, fills


def _build_inputs(x, w, slot_sizes, fills):
    tile_off = np.concatenate([[0], np.cumsum(slot_sizes)])
    tot = int(tile_off[-1])
    wt_cache = {}

    def wt_of(g):
        if g not in wt_cache:
            wtg = w[g].T.astype(ml_dtypes.bfloat16)
            wt_cache[g] = np.ascontiguousarray(
                wtg.reshape(KTILES, 128, DOUT).transpose(1, 0, 2)
            ).reshape(128, KTILES * DOUT)
        return wt_cache[g]

    xts, wts, idxs = [], [], []
    for c in range(NCORES):
        idx = np.full(tot * TILE, -1, dtype=np.int64)
        wt_c = np.zeros((len(slot_sizes), 128, KTILES * DOUT),
                        dtype=ml_dtypes.bfloat16)
        for s, piece in enumerate(fills[c]):
            if piece is None:
                continue
            sz, g, base, rows = piece
            wt_c[s] = wt_of(g)
            for ti, r in enumerate(rows):
                if r > 0:
                    o = (int(tile_off[s]) + ti) * TILE
                    idx[o:o + r] = np.arange(base + ti * TILE,
                                             base + ti * TILE + r)
        xpad = np.zeros((tot * TILE, DIN), dtype=np.float32)
        valid = idx >= 0
        xpad[valid] = x[idx[valid]]
        # [tile, j(tok), k, i(in)] -> [tile, i, k, j]
        x4 = xpad.reshape(tot, TILE, KTILES, 128)
        xt_c = np.ascontiguousarray(
            x4.transpose(0, 3, 2, 1)
        ).reshape(tot, 128, KTILES * TILE).astype(ml_dtypes.bfloat16)
        xts.append(xt_c)
        wts.append(wt_c)
        idxs.append(idx)
    return xts, wts, idxs, tot


def _build_program(key):
    slot_sizes, repeat = key
    nslots = len(slot_sizes)
    tile_off = np.concatenate([[0], np.cumsum(slot_sizes)])
    tot = int(tile_off[-1])

    nc = bacc.Bacc("TRN2", target_bir_lowering=False)
    xt = nc.dram_tensor(
        "xt", [tot, 128, KTILES * TILE], BF16, kind="ExternalInput")
    wt = nc.dram_tensor(
        "wt", [nslots, 128, KTILES * DOUT], BF16, kind="ExternalInput")
    y = nc.dram_tensor(
        "y", [tot, TILE, DOUT], F32, kind="ExternalOutput")

    with tile.TileContext(nc) as tc:
        with (
            tc.tile_pool(name="wp", bufs=4) as wp,
            tc.tile_pool(name="xp", bufs=3) as xp,
            tc.tile_pool(name="pp", bufs=8, space="PSUM") as pp,
            tc.tile_pool(name="yp", bufs=2) as yp,
        ):
            for _ in range(repeat):
                for s, sz in enumerate(slot_sizes):
                    off = int(tile_off[s])
                    xtile = xp.tile([128, sz * KTILES * TILE], BF16, tag="x")
                    nc.sync.dma_start(
                        out=xtile[:].rearrange("p (s f) -> p s f", s=sz),
                        in_=xt[off:off + sz].rearrange("s p f -> p s f"))
                    # weight in two halves so k=0..3 matmuls can start
                    # before the second half lands
                    wtile = wp.tile([128, KTILES * DOUT], BF16, tag="w")
                    half = KTILES * DOUT // 2
                    nc.sync.dma_start(out=wtile[:, :half], in_=wt[s][:, :half])
                    nc.sync.dma_start(out=wtile[:, half:], in_=wt[s][:, half:])
                    ytile = yp.tile([128, sz * DOUT], F32, tag="y")
                    for t in range(sz):
                        ps0 = pp.tile([128, 512], F32, tag="ps")
                        ps1 = pp.tile([128, 512], F32, tag="ps")
                        for k in range(KTILES):
                            lhsT = xtile[:, (t * KTILES + k) * TILE:
                                         (t * KTILES + k + 1) * TILE]
                            nc.tensor.matmul(
                                ps0[:], lhsT=lhsT,
                                rhs=wtile[:, k * DOUT:k * DOUT + 512],
                                start=(k == 0), stop=(k == KTILES - 1))
                            nc.tensor.matmul(
                                ps1[:], lhsT=lhsT,
                                rhs=wtile[:, k * DOUT + 512:k * DOUT + 1024],
                                start=(k == 0), stop=(k == KTILES - 1))
                        nc.vector.tensor_copy(
                            ytile[:, t * DOUT:t * DOUT + 512], ps0[:])
                        nc.scalar.copy(
                            ytile[:, t * DOUT + 512:t * DOUT + 1024], ps1[:])
                    # outputs ride the ACT HWDGE ring so writebacks don't
                    # FIFO-block the next slot's weight/x prefetch on SP
                    nc.scalar.dma_start(
                        out=y[off:off + sz].rearrange("s p f -> p s f"),
                        in_=ytile[:].rearrange("p (s f) -> p s f", s=sz))
    nc.compile()
    return nc


def kernel(input, weight, grouped_mm_offs):
    global LAST_EXEC_NS, SPMD_WALL_S
    x = np.ascontiguousarray(np.asarray(input, dtype=np.float32))
    w = np.ascontiguousarray(np.asarray(weight, dtype=np.float32))
    offs = np.asarray(grouped_mm_offs, dtype=np.int32)

    repeat = int(os.environ.get("KERNEL_REPEAT", "1"))
    slot_sizes, fills = _plan(offs)
    key = (tuple(slot_sizes), repeat)
    if key not in _COMPILED:
        _COMPILED[key] = _build_program(key)
    nc = _COMPILED[key]

    xts, wts, idxs, tot = _build_inputs(x, w, slot_sizes, fills)
    in_maps = [{"xt": xts[c], "wt": wts[c]} for c in range(NCORES)]
    t0 = time.time()
    res = run_bass_kernel_spmd(nc, in_maps, core_ids=list(range(NCORES)))
    SPMD_WALL_S = time.time() - t0
    LAST_EXEC_NS = res.exec_time_ns

    out = np.empty((T_TOK, DOUT), dtype=np.float32)
    for c in range(NCORES):
        rows = np.asarray(res.results[c]["y"], dtype=np.float32).reshape(
            tot * TILE, DOUT)
        valid = idxs[c] >= 0
        out[idxs[c][valid]] = rows[valid]
    return out



# revision 5
# speedup vs baseline: 1.1028x; 1.1028x over previous
"""Ragged grouped-GEMM (MoE group linear) on 8 trn2 NeuronCores.

y[s_g:e_g] = x[s_g:e_g] @ w[g].T  for 64 expert segments given by
cumulative offsets.

Strategy: token-shard 8192 tokens per core (perfectly balanced
compute).  Matmuls are weight-stationary: lhsT = a [128k x 128dout]
tile of the expert weight, rhs = x^T [128k x N tokens], out =
[128dout x N] in PSUM.  Matmul cost on the PE is proportional to the
MOVING (token) dim only, so ragged segments cost exactly their token
count -- no 128-token tile padding.  The host packs each core's
per-expert token segments into a shared static "slot" inventory
(token-granular capacities, one 2MB weight DMA per slot, tokens
processed in <=512-token PSUM chunks).  The slot inventory is
optimized for the actual offsets at compile time by a local search
and shared by all 8 cores, so a single static SPMD program serves
all cores; per-core raggedness lives entirely in the data (which
expert weight / which tokens each slot carries).  Matmuls run in
bf16 with fp32 PSUM accumulation; x is pre-transposed on the host
(feature-major per chunk) and y is written back transposed in bf16
and re-assembled on the host.
"""

import bisect
import math
import os
import time
import numpy as np
import ml_dtypes

import concourse.bass as bass
import concourse.mybir as mybir
import concourse.tile as tile
from concourse import bacc
from concourse.bass_utils import run_bass_kernel_spmd

T_TOK = 65536
G_EXP = 64
DIN = 1024
DOUT = 1024
NCORES = 8
TPC = T_TOK // NCORES
KTILES = DIN // 128
JTILES = DOUT // 128
CHUNK = 512  # PSUM bank: 512 fp32 per partition
BF16 = mybir.dt.bfloat16
F32 = mybir.dt.float32

_COMPILED = {}
LAST_EXEC_NS = None
SPMD_WALL_S = None


def _segments(offs):
    """Per-core list of (expert, tok_start, ntokens)."""
    bounds = np.concatenate([[0], np.asarray(offs, dtype=np.int64)])
    per_core = []
    for c in range(NCORES):
        lo, hi = c * TPC, (c + 1) * TPC
        segs = []
        for g in range(G_EXP):
            s, e = max(int(bounds[g]), lo), min(int(bounds[g + 1]), hi)
            if e > s:
                segs.append((g, s, e - s))
        per_core.append(segs)
    return per_core


def _fit_core(sizes, inv):
    """Greedy fit of segment sizes onto slot capacities: per segment
    (desc), smallest free slot >= remainder if any, else largest free
    (split).  Returns list of (slot_i, used) groups per segment or None."""
    free = sorted((c, i) for i, c in enumerate(inv))
    caps = [c for c, _ in free]
    out = []
    for si, n in sorted(enumerate(sizes), key=lambda t: -t[1]):
        rem = n
        grp = []
        while rem > 0:
            if not free:
                return None
            j = bisect.bisect_left(caps, rem)
            if j < len(free):
                c, i = free.pop(j)
                caps.pop(j)
                grp.append((i, rem))
                rem = 0
            else:
                c, i = free.pop()
                caps.pop()
                grp.append((i, c))
                rem -= c
        out.append((si, grp))
    return out


def _inv_cost(inv, max_slots=26):
    if len(inv) > max_slots or not inv:
        return 1e18
    cap = sum(inv)
    nch = sum((c + CHUNK - 1) // CHUNK for c in inv)
    pe = 26.67 * cap + 141 * nch
    pen = max(0.0, 6316.0 * len(inv) - 0.8 * pe) * 2
    return pe + pen


def _search_inventory(profiles, seconds=8.0):
    """Local search for a shared slot-capacity multiset minimizing the
    PE cost model, feasible for every core's segment-size profile."""
    rng = np.random.default_rng(0)

    def feasible(inv):
        return all(_fit_core(p, inv) is not None for p in profiles)

    split = []
    for segs in profiles:
        ss = []
        for n in segs:
            while n > 2048:
                ss.append(2048)
                n -= 2048
            ss.append(n)
        split.append(sorted(ss, reverse=True))
    m = max(len(s) for s in split)
    inv = [max(s[i] if i < len(s) else 0 for s in split) for i in range(m)]
    inv = [c for c in inv if c > 0]
    if not feasible(inv):  # paranoia: grow until feasible
        while not feasible(inv):
            inv = inv + [2048]
    best, bcost = list(inv), _inv_cost(inv)
    deadline = time.time() + seconds
    for seed in range(64):
        if time.time() > deadline:
            break
        cur, ccost = list(best), bcost
        for it in range(6000):
            cand = list(cur)
            move = rng.integers(0, 5)
            if move == 0 and cand:
                i = int(rng.integers(0, len(cand)))
                cand[i] -= int(rng.choice([1, 2, 4, 8, 16, 32, 64, 128, 256]))
                if cand[i] <= 0:
                    cand.pop(i)
            elif move == 1 and cand:
                cand.pop(int(rng.integers(0, len(cand))))
            elif move == 2 and cand:
                i = int(rng.integers(0, len(cand)))
                if cand[i] >= 2:
                    a = int(rng.integers(1, cand[i]))
                    cand.append(cand[i] - a)
                    cand[i] = a
            elif move == 3 and len(cand) >= 2:
                i, j = rng.choice(len(cand), 2, replace=False)
                cand[int(i)] += cand[int(j)]
                cand.pop(int(j))
            elif cand:
                i = int(rng.integers(0, len(cand)))
                cand[i] += int(rng.choice([1, 2, 4, 8, 16, 32]))
            cand = [c for c in cand if c > 0]
            if not cand or not feasible(cand):
                continue
            cc = _inv_cost(cand)
            if cc <= ccost + (50 if it < 3000 else 0):
                cur, ccost = cand, cc
                if cc < bcost:
                    best, bcost = list(cand), cc
    return sorted(best, reverse=True)


def _plan(offs):
    """Returns (slot_caps, fills): shared slot capacities (tokens, desc)
    and per-core slot fills [(expert, tok_start, n_used) or None]."""
    per_core = _segments(offs)
    profiles = [[n for _, _, n in segs] for segs in per_core]
    slot_caps = _search_inventory(profiles)

    fills = []
    for c in range(NCORES):
        segs = per_core[c]
        fit = _fit_core([n for _, _, n in segs], slot_caps)
        fill = [None] * len(slot_caps)
        for si, grp in fit:
            g, s, _ = segs[si]
            base = s
            for slot_i, used in grp:
                fill[slot_i] = (g, base, used)
                base += used
        fills.append(fill)
    return slot_caps, fills


def _chunks_of(cap):
    out = [CHUNK] * (cap // CHUNK)
    if cap % CHUNK:
        out.append(cap % CHUNK)
    return out


def _build_inputs(x, w, slot_caps, fills):
    """Host-side pack: xt [128, 8*cap] bf16 (feature-major per chunk),
    wt [nslots, 128, 8192] bf16, token index array per core."""
    cap_tot = sum(slot_caps)
    slot_off = np.concatenate([[0], np.cumsum(slot_caps)])
    wt_cache = {}

    def wt_of(g):
        if g not in wt_cache:
            wtg = w[g].T.astype(ml_dtypes.bfloat16)
            wt_cache[g] = np.ascontiguousarray(
                wtg.reshape(KTILES, 128, DOUT).transpose(1, 0, 2)
            ).reshape(128, KTILES * DOUT)
        return wt_cache[g]

    # chunk table (shared): (tok_off, n)
    chunks = []
    for s, cap in enumerate(slot_caps):
        o = int(slot_off[s])
        for n in _chunks_of(cap):
            chunks.append((o, n))
            o += n

    xts, wts, idxs = [], [], []
    for c in range(NCORES):
        idx = np.full(cap_tot, -1, dtype=np.int64)
        wt_c = np.zeros((len(slot_caps), 128, KTILES * DOUT),
                        dtype=ml_dtypes.bfloat16)
        for s, piece in enumerate(fills[c]):
            if piece is None:
                continue
            g, base, used = piece
            wt_c[s] = wt_of(g)
            o = int(slot_off[s])
            idx[o:o + used] = np.arange(base, base + used)
        xpad = np.zeros((cap_tot, DIN), dtype=ml_dtypes.bfloat16)
        valid = idx >= 0
        xpad[valid] = x[idx[valid]].astype(ml_dtypes.bfloat16)
        xt_c = np.empty((128, KTILES * cap_tot), dtype=ml_dtypes.bfloat16)
        for o, n in chunks:
            xt_c[:, KTILES * o:KTILES * (o + n)] = (
                xpad[o:o + n].reshape(n, KTILES, 128)
                .transpose(2, 1, 0).reshape(128, KTILES * n)
            )
        xts.append(xt_c)
        wts.append(wt_c)
        idxs.append(idx)
    return xts, wts, idxs, cap_tot


def _build_program(key):
    slot_caps, repeat = key
    nslots = len(slot_caps)
    cap_tot = sum(slot_caps)
    stream = KTILES * cap_tot

    nc = bacc.Bacc("TRN2", target_bir_lowering=False)
    xt = nc.dram_tensor("xt", [128, stream], BF16, kind="ExternalInput")
    wt = nc.dram_tensor(
        "wt", [nslots, 128, KTILES * DOUT], BF16, kind="ExternalInput")
    y = nc.dram_tensor("y", [128, stream], BF16, kind="ExternalOutput")

    with tile.TileContext(nc) as tc:
        with (
            tc.tile_pool(name="wp", bufs=4) as wp,
            tc.tile_pool(name="xp", bufs=3) as xp,
            tc.tile_pool(name="pp", bufs=8, space="PSUM") as pp,
            tc.tile_pool(name="yp", bufs=3) as yp,
        ):
            for r in range(repeat):
                xoff = 0
                for s, cap in enumerate(slot_caps):
                    wtile = wp.tile([128, KTILES * DOUT], BF16, tag="w")
                    chunk_list = _chunks_of(cap)
                    head = (s == 0 and r == 0)
                    if head:
                        # first slot: weight in 8 k-slices with the first
                        # x chunk DMA'd between slice 0 and 1, and the
                        # first chunk's matmuls k-outer, so the PE starts
                        # after ~1 w slice + 1 x chunk instead of the
                        # whole 2MB weight.
                        nc.sync.dma_start(
                            out=wtile[:, :DOUT], in_=wt[s][:, :DOUT])
                        n0 = chunk_list[0]
                        xtile0 = xp.tile([128, KTILES * n0], BF16, tag="x")
                        nc.sync.dma_start(
                            out=xtile0[:], in_=xt[:, xoff:xoff + KTILES * n0])
                        for k in range(1, KTILES):
                            nc.sync.dma_start(
                                out=wtile[:, k * DOUT:(k + 1) * DOUT],
                                in_=wt[s][:, k * DOUT:(k + 1) * DOUT])
                    else:
                        half = KTILES * DOUT // 2
                        nc.sync.dma_start(
                            out=wtile[:, :half], in_=wt[s][:, :half])
                        nc.sync.dma_start(
                            out=wtile[:, half:], in_=wt[s][:, half:])
                    for ci, n in enumerate(chunk_list):
                        if head and ci == 0:
                            xtile = xtile0
                        else:
                            xtile = xp.tile([128, KTILES * n], BF16, tag="x")
                            nc.sync.dma_start(
                                out=xtile[:],
                                in_=xt[:, xoff:xoff + KTILES * n])
                        ytile = yp.tile([128, JTILES * n], BF16, tag="y")
                        if head and ci == 0:
                            # k-outer: each k pass needs only w k-slice k
                            pss = [pp.tile([128, CHUNK], F32, tag="ps",
                                           name="ps")
                                   for _ in range(JTILES)]
                            for k in range(KTILES):
                                for j in range(JTILES):
                                    nc.tensor.matmul(
                                        pss[j][:, :n],
                                        lhsT=wtile[:, k * DOUT + j * 128:
                                                   k * DOUT + (j + 1) * 128],
                                        rhs=xtile[:, k * n:(k + 1) * n],
                                        start=(k == 0), stop=(k == KTILES - 1))
                            for j in range(JTILES):
                                if j % 2 == 0:
                                    nc.vector.tensor_copy(
                                        ytile[:, j * n:(j + 1) * n],
                                        pss[j][:, :n])
                                else:
                                    nc.scalar.copy(
                                        ytile[:, j * n:(j + 1) * n],
                                        pss[j][:, :n])
                        else:
                            for j in range(JTILES):
                                ps = pp.tile([128, CHUNK], F32, tag="ps")
                                for k in range(KTILES):
                                    nc.tensor.matmul(
                                        ps[:, :n],
                                        lhsT=wtile[:, k * DOUT + j * 128:
                                                   k * DOUT + (j + 1) * 128],
                                        rhs=xtile[:, k * n:(k + 1) * n],
                                        start=(k == 0), stop=(k == KTILES - 1))
                                if j % 2 == 0:
                                    nc.vector.tensor_copy(
                                        ytile[:, j * n:(j + 1) * n],
                                        ps[:, :n])
                                else:
                                    nc.scalar.copy(
                                        ytile[:, j * n:(j + 1) * n],
                                        ps[:, :n])
                        nc.scalar.dma_start(
                            out=y[:, xoff:xoff + JTILES * n], in_=ytile[:])
                        xoff += KTILES * n
    nc.compile()
    return nc


def kernel(input, weight, grouped_mm_offs):
    global LAST_EXEC_NS, SPMD_WALL_S
    x = np.ascontiguousarray(np.asarray(input, dtype=np.float32))
    w = np.ascontiguousarray(np.asarray(weight, dtype=np.float32))
    offs = np.asarray(grouped_mm_offs, dtype=np.int32)

    repeat = int(os.environ.get("KERNEL_REPEAT", "1"))
    slot_caps, fills = _plan(offs)
    key = (tuple(slot_caps), repeat)
    if key not in _COMPILED:
        _COMPILED[key] = _build_program(key)
    nc = _COMPILED[key]

    xts, wts, idxs, cap_tot = _build_inputs(x, w, slot_caps, fills)
    in_maps = [{"xt": xts[c], "wt": wts[c]} for c in range(NCORES)]
    t0 = time.time()
    res = run_bass_kernel_spmd(nc, in_maps, core_ids=list(range(NCORES)))
    SPMD_WALL_S = time.time() - t0
    LAST_EXEC_NS = res.exec_time_ns

    # decode: y[p, 8*o + j*n + t] = out[tok(o+t), j*128 + p]
    slot_off = np.concatenate([[0], np.cumsum(slot_caps)])
    chunks = []
    for s, cap in enumerate(slot_caps):
        o = int(slot_off[s])
        for n in _chunks_of(cap):
            chunks.append((o, n))
            o += n

    out = np.empty((T_TOK, DOUT), dtype=np.float32)
    for c in range(NCORES):
        yb = np.asarray(res.results[c]["y"])
        rows = np.empty((cap_tot, DOUT), dtype=np.float32)
        for o, n in chunks:
            blk = yb[:, KTILES * o:KTILES * (o + n)].reshape(128, JTILES, n)
            rows[o:o + n] = (
                blk.transpose(2, 1, 0).reshape(n, DOUT).astype(np.float32))
        valid = idxs[c] >= 0
        out[idxs[c][valid]] = rows[valid]
    return out
